# revision 6
# baseline (speedup 1.0000x reference)
"""DNC forward kernel for Trainium2, 8-core SPMD, data-parallel over batch.

Problem (hardcoded): B=16, T=512, I=256, H=512, M=2048, D=64, NH=4, O=256.
  hs = LSTM(x)                                  [B,T,H]
  keys = (hs @ W_if.T + b_if)[..., :256]        [B,T,NH,D]
  cos  = cosine_sim(keys, memory)               [B,T,NH,M]
  attn = softmax(cos, -1); reads = attn @ memory
  out  = [hs, reads] @ W_out.T + b_out          [B,T,O]

Sharding: batch 16 -> 2 rows per core; weights replicated; no collectives.

Device layout notes (per core, Bc=2, KV=T*Bc=1024; kv column index = t*2+b):
  xT       [128, 2, 1024]  bf16   x.T tiles         (k-tile of I, kv)
  whhT     [128, 4, 2048]  bf16   W_hh.T tiles      (k-tile of H, gate-dim)
  hs_sb    [128, 513, 4, 2] bf16  h in k-tile layout: partition p of slot
                                  (t,k,b) holds h_t[k*128+p] for batch b
  xg_sb    [128, 512, 16, 2] f32  precomputed x-gates, gate g = mt*128+p
  gates    PSUM [128, ngates, 2]  per gate group (i,f,g,o = mt 0-3,4-7,8-11,12-15)
  keys_sb  [64, 4, 1024]   bf16   (d, head, kv)
  exp      [128, 16, 512]  bf16   per (b,h): (slot-tile partition, slot-tile, t)
  reads    via lhsT=[memory|1] -> psum [65, 512]: rows 0-63 raw reads, row 64 sums
"""
import os
import numpy as np
import ml_dtypes

BF = ml_dtypes.bfloat16
B, T, I, H = 16, 512, 256, 512
T = int(os.environ.get("KERNEL_T", T))  # reduced-T builds for dev only
M, D, NH, O = 2048, 64, 4, 256
G = 4 * H
NCORES = 8
BC = B // NCORES
KV = T * BC
assert T % 128 == 0
CH1 = max(1, KV // 512)   # kv chunks for N<=512 matmuls
CS1 = KV // CH1
NC128 = KV // 128         # 128-wide kv chunks (norms layout)
TCT = T // 128            # 128-wide t chunks (sums layout)

_CACHE = {}


def _build_nc():
    import concourse.bass as bass  # noqa: F401
    import concourse.tile as tile
    from concourse import bacc, mybir

    f32 = mybir.dt.float32
    bf16 = mybir.dt.bfloat16
    AF = mybir.ActivationFunctionType

    nc = bacc.Bacc("TRN2", target_bir_lowering=False, debug=False,
                   enable_asserts=True, num_devices=NCORES)

    def din(name, shape, dt):
        return nc.dram_tensor(name, shape, dt, kind="ExternalInput").ap()

    d_x = din("xT", [128, 2, KV], bf16)
    d_whh = din("whhT", [128, 4, G], bf16)
    d_wih = din("wihT", [128, 2, G], bf16)
    d_wif = din("wifT", [128, 4, 256], bf16)
    d_woh = din("woutTh", [128, 4, O], bf16)
    d_wor = din("woutTr", [64, 4, O], bf16)
    d_mh = din("memhatT", [64, M], bf16)
    d_mo = din("memones", [128, 16, 65], bf16)
    d_bg = din("biasg", [128, 16], f32)
    d_bk = din("bifk", [64, 4], f32)
    d_bo = din("bout", [128, 2], f32)
    d_id = din("ident", [128, 128], f32)
    d_o64 = din("ones64", [1, 64], bf16)
    d_oc = din("onescol", [64, 1], f32)
    d_y = nc.dram_tensor("y", [BC, 2, 128, T], f32, kind="ExternalOutput").ap()

    def sb(name, shape, dt):
        return nc.alloc_sbuf_tensor(name, list(shape), dt).ap()

    s_x = sb("s_x", [128, 2, KV], bf16)
    s_whh = sb("s_whh", [128, 4, G], bf16)
    s_wih = sb("s_wih", [128, 2, G], bf16)
    s_wif = sb("s_wif", [128, 4, 256], bf16)
    s_woh = sb("s_woh", [128, 4, O], bf16)
    s_wor = sb("s_wor", [64, 4, O], bf16)
    s_mh = sb("s_mh", [64, M], bf16)
    s_mo = sb("s_mo", [128, 16, 65], bf16)
    s_bg = sb("s_bg", [128, 16], f32)
    s_bk = sb("s_bk", [64, 4], f32)
    s_bo = sb("s_bo", [128, 2], f32)
    s_id = sb("s_id", [128, 128], f32)
    s_o64 = sb("s_o64", [1, 64], bf16)
    s_oc = sb("s_oc", [64, 1], f32)

    s_xg = sb("s_xg", [128, T, 16, 2], f32)
    s_hs = sb("s_hs", [128, T + 1, 4, 2], bf16)
    s_c = sb("s_c", [128, 4, 2], f32)
    s_keys = sb("s_keys", [64, 4, KV], bf16)
    s_khat = sb("s_khat", [64, 4, KV], bf16)
    s_rkT = sb("s_rkT", [32, 128], bf16)
    s_rkf = sb("s_rkf", [1, 4096], bf16)
    s_rr = sb("s_rr", [65, 8, T], f32)      # raw reads+sum per (b,h): col bh
    s_onep = sb("s_onep", [128, 1], f32)
    s_rcT = sb("s_rcT", [32, 128], bf16)
    s_rcf = sb("s_rcf", [1, 4096], bf16)
    s_reads = sb("s_reads", [64, 4, 2, T], bf16)  # (d, head, b, t)
    s_eps = sb("s_eps", [128, 1], f32)

    # gate group -> (first mt, n mt) in pytorch i,f,g,o order; mm order g,i,f,o
    GRP = {"i": (0, 4), "f": (4, 4), "g": (8, 4), "o": (12, 4)}
    MM_ORDER = ["g", "i", "f", "o"]

    with nc.allow_low_precision("bf16 pipeline by design"), \
         tile.TileContext(nc) as tc:
        # ---- load everything ----
        for dst, src in [(s_x, d_x), (s_whh, d_whh), (s_wih, d_wih),
                         (s_wif, d_wif), (s_woh, d_woh), (s_wor, d_wor),
                         (s_mh, d_mh), (s_mo, d_mo), (s_bg, d_bg),
                         (s_bk, d_bk), (s_bo, d_bo), (s_id, d_id),
                         (s_o64, d_o64), (s_oc, d_oc)]:
            nc.sync.dma_start(dst[:], src[:])

        nc.vector.memset(s_eps[:], 1e-12)
        nc.vector.memset(s_onep[:], 1.0)

        # ---- P1: xg = x @ W_ih.T + (b_ih + b_hh), scattered to step layout
        with tc.tile_pool(name="p1ps", bufs=4, space="PSUM") as p1ps:
            for mt in range(16):
                for ch in range(CH1):
                    pg = p1ps.tile([128, CS1], f32)
                    for k in range(2):
                        nc.tensor.matmul(
                            pg[:], s_wih[:, k, mt * 128:(mt + 1) * 128],
                            s_x[:, k, ch * CS1:(ch + 1) * CS1],
                            start=(k == 0), stop=(k == 1))
                    tpc = CS1 // 2
                    dst = s_xg[:, ch * tpc:(ch + 1) * tpc, mt, :]
                    nc.vector.tensor_scalar_add(
                        dst, pg[:].rearrange("p (t b) -> p t b", b=2),
                        s_bg[:, mt:mt + 1])

        # ---- P2: LSTM recurrence ----
        nc.vector.memset(s_hs[:, 0, :, :], 0.0)
        nc.vector.memset(s_c[:], 0.0)
        with tc.tile_pool(name="psg", bufs=2, space="PSUM") as psg, \
             tc.tile_pool(name="psi", bufs=2, space="PSUM") as psi, \
             tc.tile_pool(name="psf", bufs=2, space="PSUM") as psf, \
             tc.tile_pool(name="pso", bufs=2, space="PSUM") as pso, \
             tc.tile_pool(name="sb2", bufs=3) as sb2:
            pools = {"g": psg, "i": psi, "f": psf, "o": pso}
            for t in range(T):
                pg = {}
                for gname in MM_ORDER:
                    mt0, nmt = GRP[gname]
                    pt = pools[gname].tile([128, nmt, 2], f32)
                    pg[gname] = pt
                    for j in range(nmt):
                        mt = mt0 + j
                        for k in range(4):
                            nc.tensor.matmul(
                                pt[:, j, :],
                                s_whh[:, k, mt * 128:(mt + 1) * 128],
                                s_hs[:, t, k, :],
                                start=(k == 0), stop=(k == 3))
                act = {}
                for gname in MM_ORDER:
                    mt0, nmt = GRP[gname]
                    gs = sb2.tile([128, nmt, 2], f32, tag=f"gs{gname}")
                    nc.vector.tensor_add(gs[:], pg[gname][:],
                                         s_xg[:, t, mt0:mt0 + nmt, :])
                    av = sb2.tile([128, nmt, 2], f32, tag=f"av{gname}")
                    fn = AF.Tanh if gname == "g" else AF.Sigmoid
                    nc.scalar.activation(av[:], gs[:], fn)
                    act[gname] = av
                t1 = sb2.tile([128, 4, 2], f32, tag="t1")
                nc.vector.tensor_mul(t1[:], act["i"][:], act["g"][:])
                t2 = sb2.tile([128, 4, 2], f32, tag="t2")
                nc.vector.tensor_mul(t2[:], act["f"][:], s_c[:])
                nc.vector.tensor_add(s_c[:], t1[:], t2[:])
                tch = sb2.tile([128, 4, 2], f32, tag="tch")
                nc.scalar.activation(tch[:], s_c[:], AF.Tanh)
                nc.vector.tensor_mul(s_hs[:, t + 1, :, :], act["o"][:], tch[:])

        # ---- P3: keys, norms, khat ----
        with tc.tile_pool(name="p3ps", bufs=2, space="PSUM") as p3ps, \
             tc.tile_pool(name="p3n", bufs=1, space="PSUM") as p3n, \
             tc.tile_pool(name="p3sb", bufs=2) as p3sb, \
             tc.tile_pool(name="p3t", bufs=1, space="PSUM") as p3t, \
             tc.tile_pool(name="p3b", bufs=2, space="PSUM") as p3b, \
             tc.tile_pool(name="p3rk", bufs=1) as p3rk:
            pn2 = p3n.tile([128, 32], f32)  # [kv%128, head*8 + kv//128]
            for m64 in range(4):            # head
                for ch in range(CH1):
                    tpc = CS1 // 2
                    pk = p3ps.tile([64, CS1], f32, tag="pk")
                    for k in range(4):
                        nc.tensor.matmul(
                            pk[:], s_wif[:, k, m64 * 64:(m64 + 1) * 64],
                            s_hs[:, 1 + ch * tpc: 1 + (ch + 1) * tpc, k, :],
                            start=(k == 0), stop=(k == 3))
                    nc.vector.tensor_scalar_add(
                        s_keys[0:64, m64, ch * CS1:(ch + 1) * CS1], pk[:],
                        s_bk[:, m64:m64 + 1])
                    sq = p3sb.tile([64, CS1], f32, tag="sq")
                    nc.scalar.activation(sq[:], pk[:], AF.Square,
                                         bias=s_bk[:, m64:m64 + 1])
                    for kc in range(CS1 // 128):
                        col = m64 * NC128 + ch * (CS1 // 128) + kc
                        nc.tensor.matmul(
                            pn2[:, col:col + 1],
                            sq[:, kc * 128:(kc + 1) * 128], s_oc[:],
                            start=True, stop=True)
            ncol = NH * NC128
            rka = p3rk.tile([128, ncol], f32, tag="rka")
            nc.scalar.activation(rka[:], pn2[:, 0:ncol], AF.Sqrt,
                                 bias=s_eps[:])
            rki = p3rk.tile([128, ncol], f32, tag="rki")
            nc.vector.reciprocal(rki[:], rka[:])
            ptr = p3t.tile([ncol, 128], f32)
            nc.tensor.transpose(ptr[:], rki[:], s_id[:])
            nc.vector.tensor_copy(s_rkT[0:ncol, :], ptr[:])
            nc.sync.dma_start(
                s_rkf[0:1, 0:ncol * 128].rearrange("p (r f) -> p r f", f=128),
                s_rkT[0:ncol, :])
            for h in range(NH):
                pb = p3b.tile([64, KV], f32)
                for ch in range(CH1):
                    nc.tensor.matmul(
                        pb[:, ch * CS1:(ch + 1) * CS1], s_o64[:],
                        s_rkf[0:1, h * KV + ch * CS1: h * KV + (ch + 1) * CS1],
                        start=True, stop=True)
                nc.vector.tensor_mul(s_khat[0:64, h, :], s_keys[0:64, h, :],
                                     pb[:])

        # ---- P4a: sims -> exp -> raw reads + sums ----
        khat_v = s_khat.rearrange("d h (t b) -> d h t b", b=2)
        with tc.tile_pool(name="p4sim", bufs=2, space="PSUM") as p4sim, \
             tc.tile_pool(name="p4r", bufs=2, space="PSUM") as p4r, \
             tc.tile_pool(name="p4e", bufs=2) as p4e:
            for b in range(BC):
                for h in range(NH):
                    bh = b * NH + h
                    ex = p4e.tile([128, 16, T], bf16)
                    for st in range(16):
                        psim = p4sim.tile([128, T], f32)
                        nc.tensor.matmul(
                            psim[:], s_mh[:, st * 128:(st + 1) * 128],
                            khat_v[0:64, h, :, b], start=True, stop=True)
                        nc.scalar.activation(ex[:, st, :], psim[:], AF.Exp)
                    pr = p4r.tile([65, T], f32)
                    for st in range(16):
                        nc.tensor.matmul(pr[:], s_mo[:, st, :], ex[:, st, :],
                                         start=(st == 0), stop=(st == 15))
                    nc.vector.tensor_copy(s_rr[:, bh, :], pr[:])

        # ---- P4b: reciprocal of sums, broadcast, divide ----
        with tc.tile_pool(name="p4ps", bufs=1, space="PSUM") as p4ps, \
             tc.tile_pool(name="p4g", bufs=1, space="PSUM") as p4g, \
             tc.tile_pool(name="p4sb", bufs=1) as p4sb, \
             tc.tile_pool(name="p4bb", bufs=2, space="PSUM") as p4bb:
            pn_s = p4g.tile([128, 8 * TCT], f32)
            for bh in range(2 * NH):
                for tc_ in range(TCT):
                    nc.tensor.matmul(
                        pn_s[:, bh * TCT + tc_: bh * TCT + tc_ + 1],
                        s_rr[64:65, bh, tc_ * 128:(tc_ + 1) * 128],
                        s_onep[64:65, 0:1], start=True, stop=True)
            rc = p4sb.tile([128, 8 * TCT], f32)
            nc.vector.reciprocal(rc[:], pn_s[:])
            ptc = p4ps.tile([8 * TCT, 128], f32)
            nc.tensor.transpose(ptc[:], rc[:], s_id[:])
            nc.vector.tensor_copy(s_rcT[0:8 * TCT, :], ptc[:])
            nc.sync.dma_start(
                s_rcf[0:1, 0:8 * TCT * 128].rearrange("p (r f) -> p r f",
                                                      f=128),
                s_rcT[0:8 * TCT, :])
            for b in range(BC):
                for h in range(NH):
                    bh = b * NH + h
                    pb = p4bb.tile([64, T], f32)
                    nc.tensor.matmul(pb[:], s_o64[:],
                                     s_rcf[0:1, bh * T:(bh + 1) * T],
                                     start=True, stop=True)
                    nc.vector.tensor_mul(s_reads[0:64, h, b, :],
                                         s_rr[0:64, bh, :], pb[:])

        # ---- P5: out = [hs, reads] @ W_out.T + b_out ----
        with tc.tile_pool(name="p5ps", bufs=2, space="PSUM") as p5ps, \
             tc.tile_pool(name="p5sb", bufs=2) as p5sb:
            for b in range(BC):
                for mt in range(2):
                    po = p5ps.tile([128, T], f32)
                    for k in range(4):
                        nc.tensor.matmul(
                            po[:], s_woh[:, k, mt * 128:(mt + 1) * 128],
                            s_hs[:, 1:T + 1, k, b],
                            start=(k == 0), stop=False)
                    for kh in range(4):
                        nc.tensor.matmul(
                            po[:], s_wor[:, kh, mt * 128:(mt + 1) * 128],
                            s_reads[0:64, kh, b, :],
                            start=False, stop=(kh == 3))
                    ov = p5sb.tile([128, T], f32)
                    nc.vector.tensor_scalar_add(ov[:], po[:],
                                                s_bo[:, mt:mt + 1])
                    nc.sync.dma_start(d_y[b, mt, :, :], ov[:])

    nc.compile()
    return nc


def _host_prep(inputs):
    x = np.asarray(inputs["x"], np.float32)
    memory = np.asarray(inputs["memory"], np.float32)
    W_ih = np.asarray(inputs["W_ih"], np.float32)
    W_hh = np.asarray(inputs["W_hh"], np.float32)
    b_ih = np.asarray(inputs["b_ih"], np.float32)
    b_hh = np.asarray(inputs["b_hh"], np.float32)
    W_if = np.asarray(inputs["W_if"], np.float32)
    b_if = np.asarray(inputs["b_if"], np.float32)
    W_out = np.asarray(inputs["W_out"], np.float32)
    b_out = np.asarray(inputs["b_out"], np.float32)

    def ktile(a, p=128):  # [K, N] -> [p, K//p, N]
        K, N = a.shape
        return np.ascontiguousarray(
            a.reshape(K // p, p, N).transpose(1, 0, 2))

    shared = {
        "whhT": ktile(W_hh.T).astype(BF),
        "wihT": ktile(W_ih.T).astype(BF),
        "wifT": ktile(W_if[:NH * D].T).astype(BF),
        "woutTh": ktile(W_out.T[:H]).astype(BF),
        "woutTr": ktile(W_out.T[H:H + NH * D], p=64).astype(BF),
        "memhatT": np.ascontiguousarray(
            (memory / np.linalg.norm(memory, axis=1, keepdims=True)).T
        ).astype(BF),
        "memones": ktile(np.concatenate(
            [memory, np.ones((M, 1), np.float32)], axis=1)).astype(BF),
        "biasg": np.ascontiguousarray((b_ih + b_hh).reshape(16, 128).T),
        "bifk": np.ascontiguousarray(b_if[:NH * D].reshape(4, 64).T),
        "bout": np.ascontiguousarray(b_out.reshape(2, 128).T),
        "ident": np.eye(128, dtype=np.float32),
        "ones64": np.ones((1, 64), np.float32).astype(BF),
        "onescol": np.ones((64, 1), np.float32),
    }
    in_maps = []
    for c in range(NCORES):
        xs = x[c * BC:(c + 1) * BC]              # [2, T, I]
        xT = xs.transpose(2, 1, 0).reshape(I, KV)  # col = t*2+b
        m = dict(shared)
        m["xT"] = np.ascontiguousarray(
            xT.reshape(2, 128, KV).transpose(1, 0, 2)).astype(BF)
        in_maps.append(m)
    return in_maps


def _run(inputs, trace=False, trace_kwargs=None):
    from concourse.bass_utils import run_bass_kernel_spmd
    if "nc" not in _CACHE:
        _CACHE["nc"] = _build_nc()
    nc = _CACHE["nc"]
    in_maps = _host_prep(inputs)
    kw = {}
    if trace:
        kw["trace"] = True
        kw.update(trace_kwargs or {})
    res = run_bass_kernel_spmd(nc, in_maps, list(range(NCORES)), **kw)
    y = np.empty((B, T, O), np.float32)
    for c in range(NCORES):
        yd = res.results[c]["y"]  # [BC, 2, 128, T]
        y[c * BC:(c + 1) * BC] = yd.transpose(0, 3, 1, 2).reshape(BC, T, O)
    return y, res


def kernel(**inputs):
    y, _ = _run(inputs)
    return y


# revision 7
# speedup vs baseline: 1.0004x; 1.0004x over previous
"""DNC forward kernel for Trainium2, 8-core SPMD, data-parallel over batch.

Problem (hardcoded): B=16, T=512, I=256, H=512, M=2048, D=64, NH=4, O=256.
  hs = LSTM(x)                                  [B,T,H]
  keys = (hs @ W_if.T + b_if)[..., :256]        [B,T,NH,D]
  cos  = cosine_sim(keys, memory)               [B,T,NH,M]
  attn = softmax(cos, -1); reads = attn @ memory
  out  = [hs, reads] @ W_out.T + b_out          [B,T,O]

Sharding: batch 16 -> 2 rows per core; weights replicated; no collectives.

Device layout notes (per core, Bc=2, KV=T*Bc=1024; kv column index = t*2+b):
  xT       [128, 2, 1024]  bf16   x.T tiles         (k-tile of I, kv)
  whhT     [128, 4, 2048]  bf16   W_hh.T tiles      (k-tile of H, gate-dim)
  hs_sb    [128, 513, 4, 2] bf16  h in k-tile layout: partition p of slot
                                  (t,k,b) holds h_t[k*128+p] for batch b
  xg_sb    [128, 512, 16, 2] f32  precomputed x-gates, gate g = mt*128+p
  gates    PSUM [128, ngates, 2]  per gate group (i,f,g,o = mt 0-3,4-7,8-11,12-15)
  keys_sb  [64, 4, 1024]   bf16   (d, head, kv)
  exp      [128, 16, 512]  bf16   per (b,h): (slot-tile partition, slot-tile, t)
  reads    via lhsT=[memory|1] -> psum [65, 512]: rows 0-63 raw reads, row 64 sums
"""
import os
import numpy as np
import ml_dtypes

BF = ml_dtypes.bfloat16
B, T, I, H = 16, 512, 256, 512
T = int(os.environ.get("KERNEL_T", T))  # reduced-T builds for dev only
M, D, NH, O = 2048, 64, 4, 256
G = 4 * H
NCORES = 8
BC = B // NCORES
KV = T * BC
assert T % 128 == 0
CH1 = max(1, KV // 512)   # kv chunks for N<=512 matmuls
CS1 = KV // CH1
NC128 = KV // 128         # 128-wide kv chunks (norms layout)
TCT = T // 128            # 128-wide t chunks (sums layout)

_CACHE = {}


def _build_nc():
    import concourse.bass as bass  # noqa: F401
    import concourse.tile as tile
    from concourse import bacc, mybir

    f32 = mybir.dt.float32
    bf16 = mybir.dt.bfloat16
    AF = mybir.ActivationFunctionType

    nc = bacc.Bacc("TRN2", target_bir_lowering=False, debug=False,
                   enable_asserts=True, num_devices=NCORES)

    def din(name, shape, dt):
        return nc.dram_tensor(name, shape, dt, kind="ExternalInput").ap()

    d_x = din("xT", [128, 2, KV], bf16)
    d_whh = din("whhT", [128, 4, G], bf16)
    d_wih = din("wihT", [128, 2, G], bf16)
    d_wif = din("wifT", [128, 4, 256], bf16)
    d_woh = din("woutTh", [128, 4, O], bf16)
    d_wor = din("woutTr", [64, 4, O], bf16)
    d_mh = din("memhatT", [64, M], bf16)
    d_mo = din("memones", [128, 16, 65], bf16)
    d_bg = din("biasg", [128, 16], f32)
    d_bk = din("bifk", [64, 4], f32)
    d_bo = din("bout", [128, 2], f32)
    d_id = din("ident", [128, 128], f32)
    d_o64 = din("ones64", [1, 64], bf16)
    d_oc = din("onescol", [64, 1], f32)
    d_y = nc.dram_tensor("y", [BC, 2, 128, T], f32, kind="ExternalOutput").ap()

    def sb(name, shape, dt):
        return nc.alloc_sbuf_tensor(name, list(shape), dt).ap()

    s_x = sb("s_x", [128, 2, KV], bf16)
    s_whh = sb("s_whh", [128, 4, G], bf16)
    s_wih = sb("s_wih", [128, 2, G], bf16)
    s_wif = sb("s_wif", [128, 4, 256], bf16)
    s_woh = sb("s_woh", [128, 4, O], bf16)
    s_wor = sb("s_wor", [64, 4, O], bf16)
    s_mh = sb("s_mh", [64, M], bf16)
    s_mo = sb("s_mo", [128, 16, 65], bf16)
    s_bg = sb("s_bg", [128, 16], f32)
    s_bk = sb("s_bk", [64, 4], f32)
    s_bo = sb("s_bo", [128, 2], f32)
    s_id = sb("s_id", [128, 128], f32)
    s_o64 = sb("s_o64", [1, 64], bf16)
    s_oc = sb("s_oc", [64, 1], f32)

    s_xg = sb("s_xg", [128, T, 16, 2], f32)
    s_hs = sb("s_hs", [128, T + 1, 4, 2], bf16)
    s_c = sb("s_c", [128, 4, 2], f32)
    s_keys = sb("s_keys", [64, 4, KV], bf16)
    s_khat = sb("s_khat", [64, 4, KV], bf16)
    s_rkT = sb("s_rkT", [32, 128], bf16)
    s_rkf = sb("s_rkf", [1, 4096], bf16)
    s_rr = sb("s_rr", [65, 8, T], f32)      # raw reads+sum per (b,h): col bh
    s_onep = sb("s_onep", [128, 1], f32)
    s_rcT = sb("s_rcT", [32, 128], bf16)
    s_rcf = sb("s_rcf", [1, 4096], bf16)
    s_reads = sb("s_reads", [64, 4, 2, T], bf16)  # (d, head, b, t)
    s_eps = sb("s_eps", [128, 1], f32)

    # gate group -> (first mt, n mt) in pytorch i,f,g,o order.
    # mm order: g first (tanh early), i+f merged (one add + one sigmoid), o last
    GRP = {"g": (8, 4), "if": (0, 8), "o": (12, 4)}
    MM_ORDER = ["g", "if", "o"]

    with nc.allow_low_precision("bf16 pipeline by design"), \
         tile.TileContext(nc) as tc:
        # ---- load everything ----
        for dst, src in [(s_x, d_x), (s_whh, d_whh), (s_wih, d_wih),
                         (s_wif, d_wif), (s_woh, d_woh), (s_wor, d_wor),
                         (s_mh, d_mh), (s_mo, d_mo), (s_bg, d_bg),
                         (s_bk, d_bk), (s_bo, d_bo), (s_id, d_id),
                         (s_o64, d_o64), (s_oc, d_oc)]:
            nc.sync.dma_start(dst[:], src[:])

        nc.vector.memset(s_eps[:], 1e-12)
        nc.vector.memset(s_onep[:], 1.0)

        # ---- P1: xg = x @ W_ih.T + (b_ih + b_hh), scattered to step layout
        with tc.tile_pool(name="p1ps", bufs=4, space="PSUM") as p1ps:
            for mt in range(16):
                for ch in range(CH1):
                    pg = p1ps.tile([128, CS1], f32)
                    for k in range(2):
                        nc.tensor.matmul(
                            pg[:], s_wih[:, k, mt * 128:(mt + 1) * 128],
                            s_x[:, k, ch * CS1:(ch + 1) * CS1],
                            start=(k == 0), stop=(k == 1))
                    tpc = CS1 // 2
                    dst = s_xg[:, ch * tpc:(ch + 1) * tpc, mt, :]
                    nc.vector.tensor_scalar_add(
                        dst, pg[:].rearrange("p (t b) -> p t b", b=2),
                        s_bg[:, mt:mt + 1])

        # ---- P2: LSTM recurrence ----
        nc.vector.memset(s_hs[:, 0, :, :], 0.0)
        nc.vector.memset(s_c[:], 0.0)
        with tc.tile_pool(name="psg", bufs=2, space="PSUM") as psg, \
             tc.tile_pool(name="psif", bufs=2, space="PSUM") as psif, \
             tc.tile_pool(name="pso", bufs=2, space="PSUM") as pso, \
             tc.tile_pool(name="sb2", bufs=4) as sb2:
            pools = {"g": psg, "if": psif, "o": pso}
            for t in range(T):
                pg = {}
                for gname in MM_ORDER:
                    mt0, nmt = GRP[gname]
                    pt = pools[gname].tile([128, nmt, 2], f32)
                    pg[gname] = pt
                    for j in range(nmt):
                        mt = mt0 + j
                        for k in range(4):
                            nc.tensor.matmul(
                                pt[:, j, :],
                                s_whh[:, k, mt * 128:(mt + 1) * 128],
                                s_hs[:, t, k, :],
                                start=(k == 0), stop=(k == 3))
                act = {}
                for gname in MM_ORDER:
                    mt0, nmt = GRP[gname]
                    gs = sb2.tile([128, nmt, 2], f32, tag=f"gs{gname}")
                    nc.vector.tensor_add(gs[:], pg[gname][:],
                                         s_xg[:, t, mt0:mt0 + nmt, :])
                    av = sb2.tile([128, nmt, 2], f32, tag=f"av{gname}")
                    fn = AF.Tanh if gname == "g" else AF.Sigmoid
                    nc.scalar.activation(av[:], gs[:], fn)
                    act[gname] = av
                t1 = sb2.tile([128, 4, 2], f32, tag="t1")
                nc.vector.tensor_mul(t1[:], act["if"][:, 0:4, :], act["g"][:])
                t2 = sb2.tile([128, 4, 2], f32, tag="t2")
                nc.vector.tensor_mul(t2[:], act["if"][:, 4:8, :], s_c[:])
                nc.vector.tensor_add(s_c[:], t1[:], t2[:])
                tch = sb2.tile([128, 4, 2], f32, tag="tch")
                nc.scalar.activation(tch[:], s_c[:], AF.Tanh)
                nc.vector.tensor_mul(s_hs[:, t + 1, :, :], act["o"][:], tch[:])

        # ---- P3: keys, norms, khat ----
        with tc.tile_pool(name="p3ps", bufs=2, space="PSUM") as p3ps, \
             tc.tile_pool(name="p3n", bufs=1, space="PSUM") as p3n, \
             tc.tile_pool(name="p3sb", bufs=2) as p3sb, \
             tc.tile_pool(name="p3t", bufs=1, space="PSUM") as p3t, \
             tc.tile_pool(name="p3b", bufs=2, space="PSUM") as p3b, \
             tc.tile_pool(name="p3rk", bufs=1) as p3rk:
            pn2 = p3n.tile([128, 32], f32)  # [kv%128, head*8 + kv//128]
            for m64 in range(4):            # head
                for ch in range(CH1):
                    tpc = CS1 // 2
                    pk = p3ps.tile([64, CS1], f32, tag="pk")
                    for k in range(4):
                        nc.tensor.matmul(
                            pk[:], s_wif[:, k, m64 * 64:(m64 + 1) * 64],
                            s_hs[:, 1 + ch * tpc: 1 + (ch + 1) * tpc, k, :],
                            start=(k == 0), stop=(k == 3))
                    nc.vector.tensor_scalar_add(
                        s_keys[0:64, m64, ch * CS1:(ch + 1) * CS1], pk[:],
                        s_bk[:, m64:m64 + 1])
                    sq = p3sb.tile([64, CS1], f32, tag="sq")
                    nc.scalar.activation(sq[:], pk[:], AF.Square,
                                         bias=s_bk[:, m64:m64 + 1])
                    for kc in range(CS1 // 128):
                        col = m64 * NC128 + ch * (CS1 // 128) + kc
                        nc.tensor.matmul(
                            pn2[:, col:col + 1],
                            sq[:, kc * 128:(kc + 1) * 128], s_oc[:],
                            start=True, stop=True)
            ncol = NH * NC128
            rka = p3rk.tile([128, ncol], f32, tag="rka")
            nc.scalar.activation(rka[:], pn2[:, 0:ncol], AF.Sqrt,
                                 bias=s_eps[:])
            rki = p3rk.tile([128, ncol], f32, tag="rki")
            nc.vector.reciprocal(rki[:], rka[:])
            ptr = p3t.tile([ncol, 128], f32)
            nc.tensor.transpose(ptr[:], rki[:], s_id[:])
            nc.vector.tensor_copy(s_rkT[0:ncol, :], ptr[:])
            nc.sync.dma_start(
                s_rkf[0:1, 0:ncol * 128].rearrange("p (r f) -> p r f", f=128),
                s_rkT[0:ncol, :])
            for h in range(NH):
                pb = p3b.tile([64, KV], f32)
                for ch in range(CH1):
                    nc.tensor.matmul(
                        pb[:, ch * CS1:(ch + 1) * CS1], s_o64[:],
                        s_rkf[0:1, h * KV + ch * CS1: h * KV + (ch + 1) * CS1],
                        start=True, stop=True)
                nc.vector.tensor_mul(s_khat[0:64, h, :], s_keys[0:64, h, :],
                                     pb[:])

        # ---- P4a: sims -> exp -> raw reads + sums ----
        khat_v = s_khat.rearrange("d h (t b) -> d h t b", b=2)
        with tc.tile_pool(name="p4sim", bufs=2, space="PSUM") as p4sim, \
             tc.tile_pool(name="p4r", bufs=2, space="PSUM") as p4r, \
             tc.tile_pool(name="p4e", bufs=2) as p4e:
            for b in range(BC):
                for h in range(NH):
                    bh = b * NH + h
                    ex = p4e.tile([128, 16, T], bf16)
                    for st in range(16):
                        psim = p4sim.tile([128, T], f32)
                        nc.tensor.matmul(
                            psim[:], s_mh[:, st * 128:(st + 1) * 128],
                            khat_v[0:64, h, :, b], start=True, stop=True)
                        nc.scalar.activation(ex[:, st, :], psim[:], AF.Exp)
                    pr = p4r.tile([65, T], f32)
                    for st in range(16):
                        nc.tensor.matmul(pr[:], s_mo[:, st, :], ex[:, st, :],
                                         start=(st == 0), stop=(st == 15))
                    nc.vector.tensor_copy(s_rr[:, bh, :], pr[:])

        # ---- P4b: reciprocal of sums, broadcast, divide ----
        with tc.tile_pool(name="p4ps", bufs=1, space="PSUM") as p4ps, \
             tc.tile_pool(name="p4g", bufs=1, space="PSUM") as p4g, \
             tc.tile_pool(name="p4sb", bufs=1) as p4sb, \
             tc.tile_pool(name="p4bb", bufs=2, space="PSUM") as p4bb:
            pn_s = p4g.tile([128, 8 * TCT], f32)
            for bh in range(2 * NH):
                for tc_ in range(TCT):
                    nc.tensor.matmul(
                        pn_s[:, bh * TCT + tc_: bh * TCT + tc_ + 1],
                        s_rr[64:65, bh, tc_ * 128:(tc_ + 1) * 128],
                        s_onep[64:65, 0:1], start=True, stop=True)
            rc = p4sb.tile([128, 8 * TCT], f32)
            nc.vector.reciprocal(rc[:], pn_s[:])
            ptc = p4ps.tile([8 * TCT, 128], f32)
            nc.tensor.transpose(ptc[:], rc[:], s_id[:])
            nc.vector.tensor_copy(s_rcT[0:8 * TCT, :], ptc[:])
            nc.sync.dma_start(
                s_rcf[0:1, 0:8 * TCT * 128].rearrange("p (r f) -> p r f",
                                                      f=128),
                s_rcT[0:8 * TCT, :])
            for b in range(BC):
                for h in range(NH):
                    bh = b * NH + h
                    pb = p4bb.tile([64, T], f32)
                    nc.tensor.matmul(pb[:], s_o64[:],
                                     s_rcf[0:1, bh * T:(bh + 1) * T],
                                     start=True, stop=True)
                    nc.vector.tensor_mul(s_reads[0:64, h, b, :],
                                         s_rr[0:64, bh, :], pb[:])

        # ---- P5: out = [hs, reads] @ W_out.T + b_out ----
        with tc.tile_pool(name="p5ps", bufs=2, space="PSUM") as p5ps, \
             tc.tile_pool(name="p5sb", bufs=2) as p5sb:
            for b in range(BC):
                for mt in range(2):
                    po = p5ps.tile([128, T], f32)
                    for k in range(4):
                        nc.tensor.matmul(
                            po[:], s_woh[:, k, mt * 128:(mt + 1) * 128],
                            s_hs[:, 1:T + 1, k, b],
                            start=(k == 0), stop=False)
                    for kh in range(4):
                        nc.tensor.matmul(
                            po[:], s_wor[:, kh, mt * 128:(mt + 1) * 128],
                            s_reads[0:64, kh, b, :],
                            start=False, stop=(kh == 3))
                    ov = p5sb.tile([128, T], f32)
                    nc.vector.tensor_scalar_add(ov[:], po[:],
                                                s_bo[:, mt:mt + 1])
                    nc.sync.dma_start(d_y[b, mt, :, :], ov[:])

    nc.compile()
    return nc


def _host_prep(inputs):
    x = np.asarray(inputs["x"], np.float32)
    memory = np.asarray(inputs["memory"], np.float32)
    W_ih = np.asarray(inputs["W_ih"], np.float32)
    W_hh = np.asarray(inputs["W_hh"], np.float32)
    b_ih = np.asarray(inputs["b_ih"], np.float32)
    b_hh = np.asarray(inputs["b_hh"], np.float32)
    W_if = np.asarray(inputs["W_if"], np.float32)
    b_if = np.asarray(inputs["b_if"], np.float32)
    W_out = np.asarray(inputs["W_out"], np.float32)
    b_out = np.asarray(inputs["b_out"], np.float32)

    def ktile(a, p=128):  # [K, N] -> [p, K//p, N]
        K, N = a.shape
        return np.ascontiguousarray(
            a.reshape(K // p, p, N).transpose(1, 0, 2))

    shared = {
        "whhT": ktile(W_hh.T).astype(BF),
        "wihT": ktile(W_ih.T).astype(BF),
        "wifT": ktile(W_if[:NH * D].T).astype(BF),
        "woutTh": ktile(W_out.T[:H]).astype(BF),
        "woutTr": ktile(W_out.T[H:H + NH * D], p=64).astype(BF),
        "memhatT": np.ascontiguousarray(
            (memory / np.linalg.norm(memory, axis=1, keepdims=True)).T
        ).astype(BF),
        "memones": ktile(np.concatenate(
            [memory, np.ones((M, 1), np.float32)], axis=1)).astype(BF),
        "biasg": np.ascontiguousarray((b_ih + b_hh).reshape(16, 128).T),
        "bifk": np.ascontiguousarray(b_if[:NH * D].reshape(4, 64).T),
        "bout": np.ascontiguousarray(b_out.reshape(2, 128).T),
        "ident": np.eye(128, dtype=np.float32),
        "ones64": np.ones((1, 64), np.float32).astype(BF),
        "onescol": np.ones((64, 1), np.float32),
    }
    in_maps = []
    for c in range(NCORES):
        xs = x[c * BC:(c + 1) * BC]              # [2, T, I]
        xT = xs.transpose(2, 1, 0).reshape(I, KV)  # col = t*2+b
        m = dict(shared)
        m["xT"] = np.ascontiguousarray(
            xT.reshape(2, 128, KV).transpose(1, 0, 2)).astype(BF)
        in_maps.append(m)
    return in_maps


def _run(inputs, trace=False, trace_kwargs=None):
    from concourse.bass_utils import run_bass_kernel_spmd
    if "nc" not in _CACHE:
        _CACHE["nc"] = _build_nc()
    nc = _CACHE["nc"]
    in_maps = _host_prep(inputs)
    kw = {}
    if trace:
        kw["trace"] = True
        kw.update(trace_kwargs or {})
    res = run_bass_kernel_spmd(nc, in_maps, list(range(NCORES)), **kw)
    y = np.empty((B, T, O), np.float32)
    for c in range(NCORES):
        yd = res.results[c]["y"]  # [BC, 2, 128, T]
        y[c * BC:(c + 1) * BC] = yd.transpose(0, 3, 1, 2).reshape(BC, T, O)
    return y, res


def kernel(**inputs):
    y, _ = _run(inputs)
    return y
